# revision 6
# baseline (speedup 1.0000x reference)
"""Trainium2 Bass kernel for nn_EwaldBlock (gnn_message_passing).

Strategy: shard by GRAPH (B=32 graphs -> 4 per core, batch-contiguous), so the
per-graph structure factors are computed entirely on one core and no
collective is needed.  v2 restructure vs the first working kernel:

  * MLP1 layer 2 uses per-tile stationary (silu1 tile) so its output lands
    directly in NODE-major layout -- the 16 PE transposes of xres are gone.
  * SF matmuls use trig as the stationary operand, producing sfT [2K, D]
    directly -- the per-graph sf transposes and PSUM round trips are gone.
  * LN: one batched bn_stats per 4-tile chunk, mean/var assembled with a few
    tiny DVE ops, rstd via bit-trick + Newton (no ACT Sqrt table load).
  * trig tables: Sin activations come FIRST in the ACT stream (one Sin table
    preload via a dummy op, then one Silu table load -- 2 loads total).
  * all residuals/output in bf16 (host converts back to f32); x is loaded
    only in bf16 (feature-major for MLPs, node-major for the residual into
    LN/SF).  HBM traffic ~1.9 MB read + 0.5 MB write per core.
  * inputs arrive via 8 large DMAs instead of ~20 small ones (enqueue
    instructions cost ~0.6 us each on the issuing engine).
"""

from contextlib import ExitStack

import numpy as np
import ml_dtypes

import concourse.bass as bass
import concourse.tile as tile
from concourse import mybir
from concourse.bass_utils import run_bass_kernel_spmd
from concourse.masks import make_identity

BF16 = mybir.dt.bfloat16
F32 = mybir.dt.float32
I32 = mybir.dt.int32
AF = mybir.ActivationFunctionType
ALU = mybir.AluOpType

N_CORES = 8
D = 128
K = 64
TWO_K = 2 * K
LN_EPS = 1e-5
PI = float(np.pi)
RN_C = 12582912.0          # 1.5*2^23: (x + C) - C == round-to-nearest(x), fp32
INV_2PI = float(1.0 / (2.0 * np.pi))
SQRT_MAGIC = 0x1FBD1DF5    # sqrt bit-trick seed: bits(sqrt(x)) ~ (bits(x)>>1)+M

CONFIG = {
    "act_mode": "silu",    # "silu" (HW) | "sigmoid_mul" (CoreSim-compatible)
    "split_waits": True,   # walrus needs <=1 wait/inst; CoreSim can't run nops
    "sin_clamp": False,    # CoreSim asserts |x|<=pi; HW LUT tolerates +-1e-6
}

TRACE = False            # set by test harness for profiling
LAST_EXEC_NS = None
LAST_RESULTS = None

_PROGRAM_CACHE = {}


# --------------------------------------------------------------------------
# device program
# --------------------------------------------------------------------------

def _pieces(w, maxw=512):
    p = 0
    while p < w:
        pw = min(maxw, w - p)
        yield p, pw
        p += pw


def _tile_groups(tt, size):
    out = []
    t = 0
    while t < tt:
        out.append((t, min(size, tt - t)))
        t += size
    return out


_SPLIT_TYPES = (
    "InstTensorTensor", "InstTensorScalarPtr", "InstTensorCopy",
    "InstReciprocal", "InstBNStats", "InstBNStatsAggregate",
    "InstActivation", "InstMemset", "InstIota", "InstTensorReduce",
    "InstMatmult", "InstLdweights", "InstTensorScalarAffineSelect",
    "InstCopyPredicated", "InstDMACopy", "InstDrain",
)


def _split_excess_waits(nc, limit=1):
    """walrus's per-instruction ISA structs hold few sync waits (the DVE
    TensorTensor struct rejects >1).  Move excess waits onto same-engine
    NoOps inserted immediately before the instruction."""
    n_id = 0
    for f in nc.m.functions:
        for bb in f.blocks:
            insts = bb.instructions
            out = []
            for inst in insts:
                si = inst.sync_info
                if (si is not None and si.on_wait
                        and len(si.on_wait) > limit
                        and type(inst).__name__ in _SPLIT_TYPES):
                    waits = list(si.on_wait)
                    extra, keep = waits[:-limit], waits[-limit:]
                    for wchunk in [extra[i:i + limit]
                                   for i in range(0, len(extra), limit)]:
                        nop = mybir.InstNoOp(name=f"I-waitnop-{n_id}")
                        n_id += 1
                        nop.engine = inst.engine
                        nop.sync_info = mybir.SyncInfo(
                            on_wait=list(wchunk), on_update=[])
                        out.append(nop)
                    inst.sync_info = mybir.SyncInfo(
                        on_wait=keep, on_update=list(si.on_update))
                out.append(inst)
            insts[:] = out
    return nc


def build_program(slot_T):
    """SPMD Bass program for per-core graph-slot tile counts slot_T."""
    slot_T = tuple(int(t) for t in slot_T)
    G = len(slot_T)
    TT = sum(slot_T)
    n_pad = 128 * TT
    assert G * 128 <= 512, "sf PSUM bank holds at most 4 graphs"

    kgroups = _tile_groups(TT, 8)     # trig range-reduction groups (512 cols)
    mgroups = _tile_groups(TT, 4)     # MLP chunks (512 node-cols)

    act_silu = CONFIG["act_mode"] == "silu"

    nc = bass.Bass()

    xtbf_d = nc.declare_dram_parameter("xtbf", [D, n_pad], BF16, isOutput=False)
    xnm_d = nc.declare_dram_parameter("xnm", [128, TT * D], BF16,
                                      isOutput=False)
    kdr_d = nc.declare_dram_parameter("kdr", [128, TT * K], F32, isOutput=False)
    sinc_d = nc.declare_dram_parameter("sinc", [128, TT * K], BF16,
                                       isOutput=False)
    wa_d = nc.declare_dram_parameter("wa", [D, 2 * D], BF16, isOutput=False)
    wb_d = nc.declare_dram_parameter("wb", [D, 3 * D], BF16, isOutput=False)
    out_d = nc.declare_dram_parameter("outt", [D, n_pad], BF16, isOutput=True)

    with tile.TileContext(nc) as tc, ExitStack() as ctx:
        consts = ctx.enter_context(tc.tile_pool(name="consts", bufs=1))
        pers = ctx.enter_context(tc.tile_pool(name="pers", bufs=1))
        work = ctx.enter_context(tc.tile_pool(name="work", bufs=4))
        ps = ctx.enter_context(tc.tile_pool(name="ps", bufs=4, space="PSUM"))
        trps = ctx.enter_context(tc.tile_pool(name="trps", bufs=2,
                                              space="PSUM"))
        sfps = ctx.enter_context(tc.tile_pool(name="sfps", bufs=1,
                                              space="PSUM"))

        # ---- input DMAs: few, large, earliest-needed first ---------------
        wa = consts.tile([D, 2 * D], BF16)
        nc.sync.dma_start(out=wa, in_=wa_d[:, :])
        xtbf = pers.tile([D, n_pad], BF16)
        half = (len(mgroups) // 2) * 512
        half = max(512, min(half, n_pad - 128))
        nc.sync.dma_start(out=xtbf[:, 0:half], in_=xtbf_d[:, 0:half])
        nc.sync.dma_start(out=xtbf[:, half:n_pad], in_=xtbf_d[:, half:n_pad])
        xnm_f = pers.tile([128, TT * D], BF16)
        nc.sync.dma_start(out=xnm_f, in_=xnm_d[:, :])
        wb = consts.tile([D, 3 * D], BF16)
        nc.sync.dma_start(out=wb, in_=wb_d[:, :])

        kdr_f = pers.tile([128, TT * K], F32)
        khalf = (len(kgroups) // 2) * 8 * K if len(kgroups) > 1 else TT * K
        khalf = max(K, min(khalf, TT * K))
        nc.scalar.dma_start(out=kdr_f[:, 0:khalf], in_=kdr_d[:, 0:khalf])
        if khalf < TT * K:
            nc.scalar.dma_start(out=kdr_f[:, khalf:], in_=kdr_d[:, khalf:])
        sinc_f = pers.tile([128, TT * K], BF16)
        nc.scalar.dma_start(out=sinc_f, in_=sinc_d[:, :])

        xnm = xnm_f.rearrange("p (t d) -> p t d", d=D)
        kdr = kdr_f.rearrange("p (t k) -> p t k", k=K)
        sinc = sinc_f.rearrange("p (t k) -> p t k", k=K)

        # ---- constants ---------------------------------------------------
        for i, cv in enumerate([0.0, PI / 2.0]):
            cvt = consts.tile([128, 1], F32, name=f"constap{i}")
            nc.vector.memset(cvt, cv)
            nc.const_aps.aps[(F32, float(cv))] = cvt
        zcol = nc.const_aps.aps[(F32, 0.0)]

        ident = consts.tile([D, D], BF16)
        make_identity(nc, ident)

        def act(dst, src_psum):
            if act_silu:
                nc.scalar.activation(dst, src_psum, AF.Silu)
            else:
                sg = work.tile(list(dst.shape), BF16, name="sgm", tag="sgm")
                nc.scalar.activation(sg, src_psum, AF.Sigmoid)
                nc.vector.tensor_mul(dst, src_psum, sg)

        # preload the Sin table while input DMAs are in flight
        dsin = consts.tile([128, 1], BF16)
        nc.scalar.activation(dsin, zcol, AF.Sin)

        # ---- trig: cos/sin(k_dot_r)*sinc, node-major [128, t, 2K] --------
        trig_f = pers.tile([128, TT * TWO_K], BF16)
        trig = trig_f.rearrange("p (t k) -> p t k", k=TWO_K)
        for (t0, nt) in kgroups:
            ks = kdr[:, t0:t0 + nt, :]
            k1c = work.tile([128, nt, K], F32, tag="k1c", name=f"k1c{t0}")
            nc.vector.tensor_scalar(out=k1c, in0=ks, scalar1=INV_2PI,
                                    scalar2=RN_C, op0=ALU.mult, op1=ALU.add)
            kr = work.tile([128, nt, K], F32, tag="kr", name=f"kr{t0}")
            nc.vector.tensor_scalar(out=kr, in0=k1c, scalar1=RN_C,
                                    scalar2=None, op0=ALU.subtract)
            rs = work.tile([128, nt, K], F32, tag="rs", name=f"rs{t0}")
            nc.vector.scalar_tensor_tensor(out=rs, in0=kr, scalar=-2.0 * PI,
                                           in1=ks, op0=ALU.mult, op1=ALU.add)
            rc = work.tile([128, nt, K], F32, tag="rc", name=f"rc{t0}")
            nc.vector.scalar_tensor_tensor(out=rc, in0=rs, scalar=-1.0,
                                           in1=rs, op0=ALU.mult, op1=ALU.max)
            if CONFIG["sin_clamp"]:
                rs2 = work.tile([128, nt, K], F32, tag="rs2", name=f"rs2{t0}")
                nc.vector.tensor_scalar(out=rs2, in0=rs, scalar1=PI,
                                        scalar2=-PI, op0=ALU.min, op1=ALU.max)
            else:
                rs2 = rs
            cs = work.tile([128, nt, TWO_K], BF16, tag="cs", name=f"cs{t0}")
            nc.scalar.activation(cs[:, :, 0:K], rc, AF.Sin,
                                 bias=PI / 2.0, scale=-1.0)
            nc.scalar.activation(cs[:, :, K:TWO_K], rs2, AF.Sin)
            sc = sinc[:, t0:t0 + nt, :]
            nc.vector.tensor_mul(trig[:, t0:t0 + nt, 0:K], cs[:, :, 0:K], sc)
            nc.vector.tensor_mul(trig[:, t0:t0 + nt, K:TWO_K],
                                 cs[:, :, K:TWO_K], sc)

        # preload the Silu table right after the Sins
        if act_silu:
            dsil = consts.tile([128, 1], BF16)
            nc.scalar.activation(dsil, zcol, AF.Silu)

        # ---- MLP1 layer 1 (feature-major, 512-wide) ----------------------
        s1 = pers.tile([D, n_pad], BF16)
        for (t0, nt) in mgroups:
            c0, w = 128 * t0, 128 * nt
            h1p = ps.tile([D, 512], F32, name=f"h1p{t0}", tag="ps")
            nc.tensor.matmul(h1p[:, 0:w], wa[:, 0:D], xtbf[:, c0:c0 + w],
                             start=True, stop=True)
            act(s1[:, c0:c0 + w], h1p[:, 0:w])

        # ---- trigT via PE transposes (k-major for the message matmul) ----
        trigT = pers.tile([TWO_K, n_pad], BF16)
        for (t0, nt) in mgroups:
            trp = trps.tile([TWO_K, 512], BF16, name=f"trp{t0}", tag="tr")
            for i in range(nt):
                nc.tensor.transpose(trp[:, i * 128:(i + 1) * 128],
                                    trig[:, t0 + i, :], ident)
            nc.vector.tensor_copy(trigT[:, 128 * t0:128 * (t0 + nt)],
                                  trp[:, 0:128 * nt])

        # ---- MLP1 layer 2 (node-major out) + residual + stats ------------
        # xres rows are padded to stride D+4 so batched bn_stats APs stay 3-D
        # (contiguous dims would collapse and break the per-tile grouping).
        DP = D + 4
        xres_f = pers.tile([128, TT * DP], BF16)
        xres3 = xres_f.rearrange("p (t d) -> p t d", d=DP)
        xres = xres3[:, :, 0:D]
        stats = pers.tile([128, TT, 8], F32)
        for (t0, nt) in mgroups:
            c0, w = 128 * t0, 128 * nt
            h2p = ps.tile([128, 512], F32, name=f"h2p{t0}", tag="ps")
            for i in range(nt):
                nc.tensor.matmul(h2p[:, i * 128:(i + 1) * 128],
                                 s1[:, c0 + i * 128:c0 + (i + 1) * 128],
                                 wa[:, D:2 * D], start=True, stop=True)
            h2 = work.tile([128, 512], BF16, tag="h2", name=f"h2{t0}")
            act(h2[:, 0:w], h2p[:, 0:w])
            h2v = h2.rearrange("p (t d) -> p t d", d=D)
            nc.gpsimd.tensor_add(xres[:, t0:t0 + nt, :],
                                 xnm[:, t0:t0 + nt, :], h2v[:, 0:nt, :])
            for i in range(nt):
                nc.vector.bn_stats(stats[:, t0 + i, 0:6], xres[:, t0 + i, :])

        # ---- LN stats -> mean + rstd (all-DVE, no Sqrt table) ------------
        m_e, m_o = stats[:, :, 1], stats[:, :, 4]
        cv_e, cv_o = stats[:, :, 2], stats[:, :, 5]
        mu = pers.tile([128, TT], F32)
        nc.vector.tensor_add(mu, m_e, m_o)          # 2*mean for now
        nc.vector.tensor_scalar(out=mu, in0=mu, scalar1=0.5, scalar2=None,
                                op0=ALU.mult)
        dd = pers.tile([128, TT], F32)
        nc.vector.tensor_sub(dd, m_e, m_o)
        cc = pers.tile([128, TT], F32)
        nc.vector.tensor_add(cc, cv_e, cv_o)
        nc.vector.tensor_scalar(out=cc, in0=cc, scalar1=1.0 / 128.0,
                                scalar2=LN_EPS, op0=ALU.mult, op1=ALU.add)
        nc.vector.tensor_mul(dd, dd, dd)
        var = pers.tile([128, TT], F32)
        nc.vector.scalar_tensor_tensor(out=var, in0=dd, scalar=0.25,
                                       in1=cc, op0=ALU.mult, op1=ALU.add)
        # rstd = rsqrt(var): seed = sqrt bit-trick on reciprocal, 2x Newton
        iv = pers.tile([128, TT], F32)
        nc.vector.reciprocal(iv, var)
        y = pers.tile([128, TT], F32)
        nc.vector.tensor_scalar(out=y.bitcast(I32), in0=iv.bitcast(I32),
                                scalar1=1, scalar2=None,
                                op0=ALU.arith_shift_right)
        nc.vector.tensor_scalar(out=y.bitcast(I32), in0=y.bitcast(I32),
                                scalar1=SQRT_MAGIC, scalar2=None,
                                op0=ALU.add)
        t1 = pers.tile([128, TT], F32)
        for _ in range(2):
            nc.vector.tensor_mul(t1, var, y)
            nc.vector.tensor_mul(t1, t1, y)
            nc.vector.tensor_scalar(out=t1, in0=t1, scalar1=-0.5, scalar2=1.5,
                                    op0=ALU.mult, op1=ALU.add)
            nc.vector.tensor_mul(y, y, t1)
        rstd = y

        # ---- xln (split across DVE and GpSimd) ---------------------------
        xln_f = pers.tile([128, TT * D], BF16)
        xln = xln_f.rearrange("p (t d) -> p t d", d=D)
        for t in range(TT):
            eng = nc.vector if (t % 2 == 0) else nc.gpsimd
            eng.tensor_scalar(out=xln[:, t, :], in0=xres[:, t, :],
                              scalar1=mu[:, t:t + 1],
                              scalar2=rstd[:, t:t + 1],
                              op0=ALU.subtract, op1=ALU.mult)

        # ---- SF: sfT[2k, d] per graph (trig stationary) ------------------
        slot_off = [0]
        for tj in slot_T:
            slot_off.append(slot_off[-1] + tj)
        kfr = wb[:, 2 * D:3 * D]
        sfp = sfps.tile([TWO_K, 512], F32, name="sfp", tag="sf")
        srsis = []
        for j in range(G):
            s0, Tj = slot_off[j], slot_T[j]
            for i in range(Tj):
                t = s0 + i
                nc.tensor.matmul(sfp[:, j * 128:j * 128 + D],
                                 trig[:, t, :], xln[:, t, :],
                                 start=(i == 0), stop=(i == Tj - 1))
            srsi = work.tile([TWO_K, D], BF16, tag="srsi", bufs=G,
                             name=f"srsi{j}")
            nc.vector.tensor_mul(srsi, sfp[:, j * 128:j * 128 + D], kfr)
            srsis.append(srsi)

        # ---- MSG matmuls + x2 = x + msg (bf16) ---------------------------
        x2bf = pers.tile([D, n_pad], BF16)
        for j in range(G):
            off = 128 * slot_off[j]
            for p, pw in _pieces(128 * slot_T[j]):
                mg = ps.tile([D, 512], F32, name=f"mg{j}_{p}", tag="ps")
                nc.tensor.matmul(mg[:, 0:pw], srsis[j],
                                 trigT[:, off + p:off + p + pw],
                                 start=True, stop=True)
                nc.vector.tensor_add(x2bf[:, off + p:off + p + pw],
                                     xtbf[:, off + p:off + p + pw],
                                     mg[:, 0:pw])

        # ---- MLP2 + final residual + store -------------------------------
        outb = pers.tile([D, n_pad], BF16)
        for (t0, nt) in mgroups:
            c0, w = 128 * t0, 128 * nt
            u1p = ps.tile([D, 512], F32, name=f"u1p{t0}", tag="ps")
            nc.tensor.matmul(u1p[:, 0:w], wb[:, 0:D], x2bf[:, c0:c0 + w],
                             start=True, stop=True)
            u1 = work.tile([D, 512], BF16, tag="u1", name=f"u1{t0}")
            act(u1[:, 0:w], u1p[:, 0:w])
            u2p = ps.tile([D, 512], F32, name=f"u2p{t0}", tag="ps")
            nc.tensor.matmul(u2p[:, 0:w], wb[:, D:2 * D], u1[:, 0:w],
                             start=True, stop=True)
            u2 = work.tile([D, 512], BF16, tag="u2", name=f"u2{t0}")
            act(u2[:, 0:w], u2p[:, 0:w])
            nc.vector.tensor_add(outb[:, c0:c0 + w], x2bf[:, c0:c0 + w],
                                 u2[:, 0:w])
            nc.sync.dma_start(out=out_d[:, c0:c0 + w], in_=outb[:, c0:c0 + w])

    if CONFIG["split_waits"]:
        _split_excess_waits(nc)
    return nc


# --------------------------------------------------------------------------
# host side
# --------------------------------------------------------------------------

def _shard(batch, n_graphs):
    """Graph segments + serpentine graph->core/slot assignment."""
    bounds = np.searchsorted(batch, np.arange(n_graphs + 1))
    sizes = np.diff(bounds)
    order = np.argsort(-sizes, kind="stable")
    g_per_core = n_graphs // N_CORES
    gid = np.empty((N_CORES, g_per_core), dtype=np.int64)
    for j in range(g_per_core):
        sl = order[j * N_CORES:(j + 1) * N_CORES]
        if j % 2 == 1:
            sl = sl[::-1]
        gid[:, j] = sl
    slot_T = tuple(
        max(1, int(np.ceil(max(sizes[gid[c][j]] for c in range(N_CORES)) / 128)))
        for j in range(g_per_core))
    return bounds, gid, slot_T


def kernel(x_scalar, k_dot_r, sinc_damping, batch, down_projection,
           W_pre1, W_pre2, ln_gamma, ln_beta, W_up, W_upd1, W_upd2):
    x_scalar = np.asarray(x_scalar, dtype=np.float32)
    k_dot_r = np.asarray(k_dot_r, dtype=np.float32)
    sinc_damping = np.asarray(sinc_damping, dtype=np.float32)
    batch = np.asarray(batch).astype(np.int64)
    down_projection = np.asarray(down_projection, dtype=np.float32)
    W_pre1 = np.asarray(W_pre1, dtype=np.float32)
    W_pre2 = np.asarray(W_pre2, dtype=np.float32)
    ln_gamma = np.asarray(ln_gamma, dtype=np.float32)
    ln_beta = np.asarray(ln_beta, dtype=np.float32)
    W_up = np.asarray(W_up, dtype=np.float32)
    W_upd1 = np.asarray(W_upd1, dtype=np.float32)
    W_upd2 = np.asarray(W_upd2, dtype=np.float32)

    assert np.allclose(ln_beta, 0.0), "nonzero ln_beta not supported"

    n, d = x_scalar.shape
    n_graphs = int(batch.max()) + 1 if batch.size else 1
    n_graphs = max(n_graphs, N_CORES)
    while n_graphs % N_CORES:
        n_graphs += 1

    bounds, gid, slot_T = _shard(batch, n_graphs)
    g_per_core = n_graphs // N_CORES
    TT = sum(slot_T)
    n_pad = 128 * TT
    offs = np.cumsum([0] + [128 * t for t in slot_T])

    key = (slot_T, CONFIG["act_mode"], CONFIG["split_waits"],
           CONFIG["sin_clamp"])
    if key not in _PROGRAM_CACHE:
        _PROGRAM_CACHE[key] = build_program(slot_T)
    nc = _PROGRAM_CACHE[key]

    bf = ml_dtypes.bfloat16
    # kfilter with gamma folded, replicated for the cos and sin halves
    kf = down_projection @ (W_up * ln_gamma[:, None]).T        # [K, D]
    kfr = np.concatenate([kf, kf], axis=0)                     # [2K, D]
    shared = {
        "wa": np.ascontiguousarray(
            np.concatenate([W_pre1.T, W_pre2.T], axis=1)).astype(bf),
        "wb": np.ascontiguousarray(
            np.concatenate([W_upd1.T, W_upd2.T, kfr], axis=1)).astype(bf),
    }

    in_maps = []
    for c in range(N_CORES):
        xp = np.zeros((n_pad, D), np.float32)
        kdrp = np.zeros((n_pad, K), np.float32)
        sincp = np.zeros((n_pad, K), np.float32)
        for j in range(g_per_core):
            g = gid[c][j]
            s, e = bounds[g], bounds[g + 1]
            xp[offs[j]:offs[j] + e - s] = x_scalar[s:e]
            kdrp[offs[j]:offs[j] + e - s] = k_dot_r[s:e]
            sincp[offs[j]:offs[j] + e - s] = sinc_damping[s:e]

        # node-major [n_pad, F] -> per-tile [128, T*F] shuffled layout
        def shuf(a):
            f = a.shape[1]
            blk = a.reshape(TT, 128, f)
            return np.ascontiguousarray(
                np.transpose(blk, (1, 0, 2)).reshape(128, TT * f))

        xt = np.ascontiguousarray(xp.T)
        in_maps.append(dict(shared,
                            xtbf=xt.astype(bf),
                            xnm=shuf(xp).astype(bf),
                            kdr=shuf(kdrp),
                            sinc=shuf(sincp).astype(bf)))

    global LAST_EXEC_NS, LAST_RESULTS
    res = run_bass_kernel_spmd(nc, in_maps, list(range(N_CORES)), trace=TRACE)
    LAST_RESULTS = res
    LAST_EXEC_NS = getattr(res, "exec_time_ns", None)
    out = np.zeros((n, d), np.float32)
    for c in range(N_CORES):
        outT = np.asarray(res.results[c]["outt"]).astype(np.float32)
        for j in range(g_per_core):
            g = gid[c][j]
            s, e = bounds[g], bounds[g + 1]
            out[s:e] = outT[:, offs[j]:offs[j] + e - s].T
    return out


# revision 15
# speedup vs baseline: 1.1422x; 1.1422x over previous
"""Trainium2 Bass kernel for nn_EwaldBlock (gnn_message_passing).

Strategy: shard by GRAPH (B=32 graphs -> 4 per core, batch-contiguous), so the
per-graph structure factors are computed entirely on one core and no
collective is needed.  v2 restructure vs the first working kernel:

  * MLP1 layer 2 uses per-tile stationary (silu1 tile) so its output lands
    directly in NODE-major layout -- the 16 PE transposes of xres are gone.
  * SF matmuls use trig as the stationary operand, producing sfT [2K, D]
    directly -- the per-graph sf transposes and PSUM round trips are gone.
  * LN: one batched bn_stats per 4-tile chunk, mean/var assembled with a few
    tiny DVE ops, rstd via bit-trick + Newton (no ACT Sqrt table load).
  * trig tables: Sin activations come FIRST in the ACT stream (one Sin table
    preload via a dummy op, then one Silu table load -- 2 loads total).
  * all residuals/output in bf16 (host converts back to f32); x is loaded
    only in bf16 (feature-major for MLPs, node-major for the residual into
    LN/SF).  HBM traffic ~1.9 MB read + 0.5 MB write per core.
  * inputs arrive via 8 large DMAs instead of ~20 small ones (enqueue
    instructions cost ~0.6 us each on the issuing engine).
"""

from contextlib import ExitStack

import numpy as np
import ml_dtypes

import concourse.bass as bass
import concourse.tile as tile
from concourse import mybir
from concourse.bass_utils import run_bass_kernel_spmd
from concourse.masks import make_identity

BF16 = mybir.dt.bfloat16
F32 = mybir.dt.float32
I32 = mybir.dt.int32
AF = mybir.ActivationFunctionType
ALU = mybir.AluOpType

N_CORES = 8
D = 128
K = 64
TWO_K = 2 * K
LN_EPS = 1e-5
PI = float(np.pi)
RN_C = 12582912.0          # 1.5*2^23: (x + C) - C == round-to-nearest(x), fp32
INV_2PI = float(1.0 / (2.0 * np.pi))
SQRT_MAGIC = 0x1FBD1DF5    # sqrt bit-trick seed: bits(sqrt(x)) ~ (bits(x)>>1)+M

CONFIG = {
    "act_mode": "silu",    # "silu" (HW) | "sigmoid_mul" (CoreSim-compatible)
    "split_waits": True,   # walrus needs <=1 wait/inst; CoreSim can't run nops
    "sin_clamp": False,    # CoreSim asserts |x|<=pi; HW LUT tolerates +-1e-6
}

TRACE = False            # set by test harness for profiling
LAST_EXEC_NS = None
LAST_RESULTS = None

_PROGRAM_CACHE = {}


# --------------------------------------------------------------------------
# device program
# --------------------------------------------------------------------------

def _pieces(w, maxw=512):
    p = 0
    while p < w:
        pw = min(maxw, w - p)
        yield p, pw
        p += pw


def _tile_groups(tt, size):
    out = []
    t = 0
    while t < tt:
        out.append((t, min(size, tt - t)))
        t += size
    return out


_SPLIT_TYPES = (
    "InstTensorTensor", "InstTensorScalarPtr", "InstTensorCopy",
    "InstReciprocal", "InstBNStats", "InstBNStatsAggregate",
    "InstActivation", "InstMemset", "InstIota", "InstTensorReduce",
    "InstMatmult", "InstLdweights", "InstTensorScalarAffineSelect",
    "InstCopyPredicated", "InstDMACopy", "InstDrain",
)


def _split_excess_waits(nc, limit=1):
    """walrus's per-instruction ISA structs hold few sync waits (the DVE
    TensorTensor struct rejects >1).  Move excess waits onto same-engine
    NoOps inserted immediately before the instruction."""
    n_id = 0
    for f in nc.m.functions:
        for bb in f.blocks:
            insts = bb.instructions
            out = []
            for inst in insts:
                si = inst.sync_info
                if (si is not None and si.on_wait
                        and len(si.on_wait) > limit
                        and type(inst).__name__ in _SPLIT_TYPES):
                    waits = list(si.on_wait)
                    extra, keep = waits[:-limit], waits[-limit:]
                    for wchunk in [extra[i:i + limit]
                                   for i in range(0, len(extra), limit)]:
                        nop = mybir.InstNoOp(name=f"I-waitnop-{n_id}")
                        n_id += 1
                        nop.engine = inst.engine
                        nop.sync_info = mybir.SyncInfo(
                            on_wait=list(wchunk), on_update=[])
                        out.append(nop)
                    inst.sync_info = mybir.SyncInfo(
                        on_wait=keep, on_update=list(si.on_update))
                out.append(inst)
            insts[:] = out
    return nc


def build_program(slot_T):
    """SPMD Bass program for per-core graph-slot tile counts slot_T."""
    slot_T = tuple(int(t) for t in slot_T)
    G = len(slot_T)
    TT = sum(slot_T)
    n_pad = 128 * TT
    assert G * 128 <= 512, "sf PSUM bank holds at most 4 graphs"

    kgroups = _tile_groups(TT, 8)     # trig range-reduction groups (512 cols)
    mgroups = _tile_groups(TT, 4)     # MLP chunks (512 node-cols)

    act_silu = CONFIG["act_mode"] == "silu"

    nc = bass.Bass()

    xtbf_d = nc.declare_dram_parameter("xtbf", [D, n_pad], BF16, isOutput=False)
    xnm_d = nc.declare_dram_parameter("xnm", [128, TT * D], BF16,
                                      isOutput=False)
    kdr_d = nc.declare_dram_parameter("kdr", [128, TT * K], F32, isOutput=False)
    sinc_d = nc.declare_dram_parameter("sinc", [128, TT * TWO_K], BF16,
                                       isOutput=False)
    wa_d = nc.declare_dram_parameter("wa", [D, 2 * D], BF16, isOutput=False)
    wb_d = nc.declare_dram_parameter("wb", [D, 3 * D], BF16, isOutput=False)
    out_d = nc.declare_dram_parameter("outt", [D, n_pad], BF16, isOutput=True)

    with tile.TileContext(nc) as tc, ExitStack() as ctx:
        consts = ctx.enter_context(tc.tile_pool(name="consts", bufs=1))
        pers = ctx.enter_context(tc.tile_pool(name="pers", bufs=1))
        work = ctx.enter_context(tc.tile_pool(name="work", bufs=4))
        ps = ctx.enter_context(tc.tile_pool(name="ps", bufs=4, space="PSUM"))
        trps = ctx.enter_context(tc.tile_pool(name="trps", bufs=2,
                                              space="PSUM"))
        sfps = ctx.enter_context(tc.tile_pool(name="sfps", bufs=1,
                                              space="PSUM"))

        # ---- input DMAs: few, large, earliest-needed first ---------------
        wa = consts.tile([D, 2 * D], BF16)
        nc.sync.dma_start(out=wa, in_=wa_d[:, :])
        xtbf = pers.tile([D, n_pad], BF16)
        half = (len(mgroups) // 2) * 512
        half = max(512, min(half, n_pad - 128))
        nc.sync.dma_start(out=xtbf[:, 0:half], in_=xtbf_d[:, 0:half])
        nc.sync.dma_start(out=xtbf[:, half:n_pad], in_=xtbf_d[:, half:n_pad])
        xnm_f = pers.tile([128, TT * D], BF16)
        nc.sync.dma_start(out=xnm_f, in_=xnm_d[:, :])
        wb = consts.tile([D, 3 * D], BF16)
        nc.sync.dma_start(out=wb, in_=wb_d[:, :])

        kdr_f = pers.tile([128, TT * K], F32)
        khalf = (len(kgroups) // 2) * 8 * K if len(kgroups) > 1 else TT * K
        khalf = max(K, min(khalf, TT * K))
        nc.scalar.dma_start(out=kdr_f[:, 0:khalf], in_=kdr_d[:, 0:khalf])
        if khalf < TT * K:
            nc.scalar.dma_start(out=kdr_f[:, khalf:], in_=kdr_d[:, khalf:])
        sinc_f = pers.tile([128, TT * TWO_K], BF16)
        nc.scalar.dma_start(out=sinc_f, in_=sinc_d[:, :])

        xnm = xnm_f.rearrange("p (t d) -> p t d", d=D)
        kdr = kdr_f.rearrange("p (t k) -> p t k", k=K)
        sinc = sinc_f.rearrange("p (t k) -> p t k", k=TWO_K)  # duplicated k

        # ---- constants ---------------------------------------------------
        for i, cv in enumerate([0.0, PI / 2.0]):
            cvt = consts.tile([128, 1], F32, name=f"constap{i}")
            nc.vector.memset(cvt, cv)
            nc.const_aps.aps[(F32, float(cv))] = cvt
        zcol = nc.const_aps.aps[(F32, 0.0)]

        ident = consts.tile([D, D], BF16)
        make_identity(nc, ident)

        def act(dst, src_psum):
            if act_silu:
                nc.scalar.activation(dst, src_psum, AF.Silu)
            else:
                sg = work.tile(list(dst.shape), BF16, name="sgm", tag="sgm")
                nc.scalar.activation(sg, src_psum, AF.Sigmoid)
                nc.vector.tensor_mul(dst, src_psum, sg)

        # preload the Sin table while input DMAs are in flight
        dsin = consts.tile([128, 1], BF16)
        nc.scalar.activation(dsin, zcol, AF.Sin)

        # ---- trig: cos/sin(k_dot_r)*sinc, node-major [128, t, 2K] --------
        trig_f = pers.tile([128, TT * TWO_K], BF16)
        trig = trig_f.rearrange("p (t k) -> p t k", k=TWO_K)
        for (t0, nt) in kgroups:
            ks = kdr[:, t0:t0 + nt, :]
            k1c = work.tile([128, nt, K], F32, tag="k1c", name=f"k1c{t0}")
            nc.vector.tensor_scalar(out=k1c, in0=ks, scalar1=INV_2PI,
                                    scalar2=RN_C, op0=ALU.mult, op1=ALU.add)
            kr = work.tile([128, nt, K], F32, tag="kr", name=f"kr{t0}")
            nc.vector.tensor_scalar(out=kr, in0=k1c, scalar1=RN_C,
                                    scalar2=None, op0=ALU.subtract)
            rs = work.tile([128, nt, K], F32, tag="rs", name=f"rs{t0}")
            nc.vector.scalar_tensor_tensor(out=rs, in0=kr, scalar=-2.0 * PI,
                                           in1=ks, op0=ALU.mult, op1=ALU.add)
            rc = work.tile([128, nt, K], F32, tag="rc", name=f"rc{t0}")
            nc.vector.scalar_tensor_tensor(out=rc, in0=rs, scalar=-1.0,
                                           in1=rs, op0=ALU.mult, op1=ALU.max)
            if CONFIG["sin_clamp"]:
                rs2 = work.tile([128, nt, K], F32, tag="rs2", name=f"rs2{t0}")
                nc.vector.tensor_scalar(out=rs2, in0=rs, scalar1=PI,
                                        scalar2=-PI, op0=ALU.min, op1=ALU.max)
            else:
                rs2 = rs
            cs = work.tile([128, nt, TWO_K], BF16, tag="cs", name=f"cs{t0}")
            nc.scalar.activation(cs[:, :, 0:K], rc, AF.Sin,
                                 bias=PI / 2.0, scale=-1.0)
            nc.scalar.activation(cs[:, :, K:TWO_K], rs2, AF.Sin)
            nc.vector.tensor_mul(trig[:, t0:t0 + nt, :], cs,
                                 sinc[:, t0:t0 + nt, :])

        # preload the Silu table right after the Sins
        if act_silu:
            dsil = consts.tile([128, 1], BF16)
            nc.scalar.activation(dsil, zcol, AF.Silu)

        # ---- MLP1 layer 1 (feature-major, 512-wide) ----------------------
        s1 = pers.tile([D, n_pad], BF16)
        for (t0, nt) in mgroups:
            c0, w = 128 * t0, 128 * nt
            h1p = ps.tile([D, 512], F32, name=f"h1p{t0}", tag="ps")
            nc.tensor.matmul(h1p[:, 0:w], wa[:, 0:D], xtbf[:, c0:c0 + w],
                             start=True, stop=True)
            act(s1[:, c0:c0 + w], h1p[:, 0:w])

        # ---- trigT via PE transposes (k-major for the message matmul) ----
        # PSUM -> SBUF copies ride the idle ACT engine, not DVE
        trigT = pers.tile([TWO_K, n_pad], BF16)
        for (t0, nt) in mgroups:
            trp = trps.tile([TWO_K, 512], BF16, name=f"trp{t0}", tag="tr")
            for i in range(nt):
                nc.tensor.transpose(trp[:, i * 128:(i + 1) * 128],
                                    trig[:, t0 + i, :], ident)
            nc.scalar.copy(trigT[:, 128 * t0:128 * (t0 + nt)],
                           trp[:, 0:128 * nt])

        # ---- MLP1 layer 2 (node-major out) + residual + stats ------------
        xres_f = pers.tile([128, TT * D], BF16)
        xres = xres_f.rearrange("p (t d) -> p t d", d=D)
        stats = pers.tile([128, TT, 6], F32)
        for (t0, nt) in mgroups:
            c0, w = 128 * t0, 128 * nt
            h2p = ps.tile([128, 512], F32, name=f"h2p{t0}", tag="ps")
            for i in range(nt):
                nc.tensor.matmul(h2p[:, i * 128:(i + 1) * 128],
                                 s1[:, c0 + i * 128:c0 + (i + 1) * 128],
                                 wa[:, D:2 * D], start=True, stop=True)
            h2 = work.tile([128, 512], BF16, tag="h2", name=f"h2{t0}")
            act(h2[:, 0:w], h2p[:, 0:w])
            h2v = h2.rearrange("p (t d) -> p t d", d=D)
            nc.gpsimd.tensor_add(xres[:, t0:t0 + nt, :],
                                 xnm[:, t0:t0 + nt, :], h2v[:, 0:nt, :])
            for i in range(nt):
                nc.vector.bn_stats(stats[:, t0 + i, :], xres[:, t0 + i, :])

        # ---- LN stats -> mean + rstd (all-DVE, no Sqrt table) ------------
        m_e, m_o = stats[:, :, 1], stats[:, :, 4]
        cv_e, cv_o = stats[:, :, 2], stats[:, :, 5]
        mu = pers.tile([128, TT], F32)
        nc.vector.tensor_add(mu, m_e, m_o)          # 2*mean for now
        nc.vector.tensor_scalar(out=mu, in0=mu, scalar1=0.5, scalar2=None,
                                op0=ALU.mult)
        dd = pers.tile([128, TT], F32)
        nc.vector.tensor_sub(dd, m_e, m_o)
        cc = pers.tile([128, TT], F32)
        nc.vector.tensor_add(cc, cv_e, cv_o)
        nc.vector.tensor_scalar(out=cc, in0=cc, scalar1=1.0 / 128.0,
                                scalar2=LN_EPS, op0=ALU.mult, op1=ALU.add)
        nc.vector.tensor_mul(dd, dd, dd)
        var = pers.tile([128, TT], F32)
        nc.vector.scalar_tensor_tensor(out=var, in0=dd, scalar=0.25,
                                       in1=cc, op0=ALU.mult, op1=ALU.add)
        # rstd = rsqrt(var): seed = sqrt bit-trick on reciprocal, 2x Newton
        iv = pers.tile([128, TT], F32)
        nc.vector.reciprocal(iv, var)
        y = pers.tile([128, TT], F32)
        nc.vector.tensor_scalar(out=y.bitcast(I32), in0=iv.bitcast(I32),
                                scalar1=1, scalar2=None,
                                op0=ALU.arith_shift_right)
        nc.vector.tensor_scalar(out=y.bitcast(I32), in0=y.bitcast(I32),
                                scalar1=SQRT_MAGIC, scalar2=None,
                                op0=ALU.add)
        t1 = pers.tile([128, TT], F32)
        for _ in range(1):
            nc.vector.tensor_mul(t1, var, y)
            nc.vector.tensor_mul(t1, t1, y)
            nc.vector.tensor_scalar(out=t1, in0=t1, scalar1=-0.5, scalar2=1.5,
                                    op0=ALU.mult, op1=ALU.add)
            nc.vector.tensor_mul(y, y, t1)
        rstd = y

        # ---- xln (DVE only: gpsimd is ~6x slower on PTR-scalar ops) ------
        xln_f = pers.tile([128, TT * D], BF16)
        xln = xln_f.rearrange("p (t d) -> p t d", d=D)
        for t in range(TT):
            nc.vector.tensor_scalar(out=xln[:, t, :], in0=xres[:, t, :],
                                    scalar1=mu[:, t:t + 1],
                                    scalar2=rstd[:, t:t + 1],
                                    op0=ALU.subtract, op1=ALU.mult)

        # ---- SF: sfT[2k, d] per graph (trig stationary) ------------------
        slot_off = [0]
        for tj in slot_T:
            slot_off.append(slot_off[-1] + tj)
        kfr = wb[:, 2 * D:3 * D]
        sfp = sfps.tile([TWO_K, 512], F32, name="sfp", tag="sf")
        srsis = []
        for j in range(G):
            s0, Tj = slot_off[j], slot_T[j]
            for i in range(Tj):
                t = s0 + i
                nc.tensor.matmul(sfp[:, j * 128:j * 128 + D],
                                 trig[:, t, :], xln[:, t, :],
                                 start=(i == 0), stop=(i == Tj - 1))
            srsi = work.tile([TWO_K, D], BF16, tag="srsi", bufs=G,
                             name=f"srsi{j}")
            nc.vector.tensor_mul(srsi, sfp[:, j * 128:j * 128 + D], kfr)
            srsis.append(srsi)

        # ---- MSG matmuls + x2 = x + msg (bf16) ---------------------------
        x2bf = pers.tile([D, n_pad], BF16)
        for j in range(G):
            off = 128 * slot_off[j]
            for p, pw in _pieces(128 * slot_T[j]):
                mg = ps.tile([D, 512], F32, name=f"mg{j}_{p}", tag="ps")
                nc.tensor.matmul(mg[:, 0:pw], srsis[j],
                                 trigT[:, off + p:off + p + pw],
                                 start=True, stop=True)
                nc.vector.tensor_add(x2bf[:, off + p:off + p + pw],
                                     xtbf[:, off + p:off + p + pw],
                                     mg[:, 0:pw])

        # ---- MLP2 + final residual + store -------------------------------
        outb = pers.tile([D, n_pad], BF16)
        for (t0, nt) in mgroups:
            c0, w = 128 * t0, 128 * nt
            u1p = ps.tile([D, 512], F32, name=f"u1p{t0}", tag="ps")
            nc.tensor.matmul(u1p[:, 0:w], wb[:, 0:D], x2bf[:, c0:c0 + w],
                             start=True, stop=True)
            u1 = work.tile([D, 512], BF16, tag="u1", name=f"u1{t0}")
            act(u1[:, 0:w], u1p[:, 0:w])
            u2p = ps.tile([D, 512], F32, name=f"u2p{t0}", tag="ps")
            nc.tensor.matmul(u2p[:, 0:w], wb[:, D:2 * D], u1[:, 0:w],
                             start=True, stop=True)
            u2 = work.tile([D, 512], BF16, tag="u2", name=f"u2{t0}")
            act(u2[:, 0:w], u2p[:, 0:w])
            nc.gpsimd.tensor_add(outb[:, c0:c0 + w], x2bf[:, c0:c0 + w],
                                 u2[:, 0:w])
            nc.sync.dma_start(out=out_d[:, c0:c0 + w], in_=outb[:, c0:c0 + w])

    if CONFIG["split_waits"]:
        _split_excess_waits(nc)
    return nc


# --------------------------------------------------------------------------
# host side
# --------------------------------------------------------------------------

def _shard(batch, n_graphs):
    """Graph segments + serpentine graph->core/slot assignment."""
    bounds = np.searchsorted(batch, np.arange(n_graphs + 1))
    sizes = np.diff(bounds)
    order = np.argsort(-sizes, kind="stable")
    g_per_core = n_graphs // N_CORES
    gid = np.empty((N_CORES, g_per_core), dtype=np.int64)
    for j in range(g_per_core):
        sl = order[j * N_CORES:(j + 1) * N_CORES]
        if j % 2 == 1:
            sl = sl[::-1]
        gid[:, j] = sl
    slot_T = tuple(
        max(1, int(np.ceil(max(sizes[gid[c][j]] for c in range(N_CORES)) / 128)))
        for j in range(g_per_core))
    return bounds, gid, slot_T


def kernel(x_scalar, k_dot_r, sinc_damping, batch, down_projection,
           W_pre1, W_pre2, ln_gamma, ln_beta, W_up, W_upd1, W_upd2):
    x_scalar = np.asarray(x_scalar, dtype=np.float32)
    k_dot_r = np.asarray(k_dot_r, dtype=np.float32)
    sinc_damping = np.asarray(sinc_damping, dtype=np.float32)
    batch = np.asarray(batch).astype(np.int64)
    down_projection = np.asarray(down_projection, dtype=np.float32)
    W_pre1 = np.asarray(W_pre1, dtype=np.float32)
    W_pre2 = np.asarray(W_pre2, dtype=np.float32)
    ln_gamma = np.asarray(ln_gamma, dtype=np.float32)
    ln_beta = np.asarray(ln_beta, dtype=np.float32)
    W_up = np.asarray(W_up, dtype=np.float32)
    W_upd1 = np.asarray(W_upd1, dtype=np.float32)
    W_upd2 = np.asarray(W_upd2, dtype=np.float32)

    assert np.allclose(ln_beta, 0.0), "nonzero ln_beta not supported"

    n, d = x_scalar.shape
    n_graphs = int(batch.max()) + 1 if batch.size else 1
    n_graphs = max(n_graphs, N_CORES)
    while n_graphs % N_CORES:
        n_graphs += 1

    bounds, gid, slot_T = _shard(batch, n_graphs)
    g_per_core = n_graphs // N_CORES
    TT = sum(slot_T)
    n_pad = 128 * TT
    offs = np.cumsum([0] + [128 * t for t in slot_T])

    key = (slot_T, CONFIG["act_mode"], CONFIG["split_waits"],
           CONFIG["sin_clamp"])
    if key not in _PROGRAM_CACHE:
        _PROGRAM_CACHE[key] = build_program(slot_T)
    nc = _PROGRAM_CACHE[key]

    bf = ml_dtypes.bfloat16
    # kfilter with gamma folded, replicated for the cos and sin halves
    kf = down_projection @ (W_up * ln_gamma[:, None]).T        # [K, D]
    kfr = np.concatenate([kf, kf], axis=0)                     # [2K, D]
    shared = {
        "wa": np.ascontiguousarray(
            np.concatenate([W_pre1.T, W_pre2.T], axis=1)).astype(bf),
        "wb": np.ascontiguousarray(
            np.concatenate([W_upd1.T, W_upd2.T, kfr], axis=1)).astype(bf),
    }

    in_maps = []
    for c in range(N_CORES):
        xp = np.zeros((n_pad, D), np.float32)
        kdrp = np.zeros((n_pad, K), np.float32)
        sincp = np.zeros((n_pad, K), np.float32)
        for j in range(g_per_core):
            g = gid[c][j]
            s, e = bounds[g], bounds[g + 1]
            xp[offs[j]:offs[j] + e - s] = x_scalar[s:e]
            kdrp[offs[j]:offs[j] + e - s] = k_dot_r[s:e]
            sincp[offs[j]:offs[j] + e - s] = sinc_damping[s:e]

        # node-major [n_pad, F] -> per-tile [128, T*F] shuffled layout
        def shuf(a, dup=False):
            f = a.shape[1]
            blk = np.transpose(a.reshape(TT, 128, f), (1, 0, 2))
            if dup:                      # duplicate k-cols for cos|sin halves
                blk = np.concatenate([blk, blk], axis=2)
                f *= 2
            return np.ascontiguousarray(blk.reshape(128, TT * f))

        xt = np.ascontiguousarray(xp.T)
        in_maps.append(dict(shared,
                            xtbf=xt.astype(bf),
                            xnm=shuf(xp).astype(bf),
                            kdr=shuf(kdrp),
                            sinc=shuf(sincp, dup=True).astype(bf)))

    global LAST_EXEC_NS, LAST_RESULTS
    res = run_bass_kernel_spmd(nc, in_maps, list(range(N_CORES)), trace=TRACE)
    LAST_RESULTS = res
    LAST_EXEC_NS = getattr(res, "exec_time_ns", None)
    out = np.zeros((n, d), np.float32)
    for c in range(N_CORES):
        outT = np.asarray(res.results[c]["outt"]).astype(np.float32)
        for j in range(g_per_core):
            g = gid[c][j]
            s, e = bounds[g], bounds[g + 1]
            out[s:e] = outT[:, offs[j]:offs[j] + e - s].T
    return out
